# revision 2
# baseline (speedup 1.0000x reference)
"""Multi-head causal attention (B=2,S=2048,D=768,H=12) on 8 NeuronCores.

Sharding: core = (batch, head_group) with 2 batches x 4 head groups of 3
heads.  Each core computes q/k/v projections for its heads, causal
attention, and a partial output projection (wo rows for its heads); the
host sums the 4 partials per batch and adds bo + bv@wo (the v-bias rides
through attention as an exact constant since softmax rows sum to 1).

Schedule (all f16 matmuls, f32 psum):
  - x shipped pre-transposed and chunk-major; per-block DMAs are
    contiguous, weights land first so the first projection starts ~4us in.
  - 8 dummy warm-up matmuls during the DMA fill keep the PE HAM activity
    window busy so the real stream starts at the full 2.4 GHz clock.
  - q/k projections packed as 3 128-column chunks [h0q|h1q], [h0k|h1k],
    [h2q|h2k]; one small on-chip DMA re-aligns h2k to partitions 0:64 so
    the h2 QK^T operands share a partition range (no duplicate columns).
  - attention per 512-query block, batched phases (dense PE bursts keep
    the clock gate warm): QK^T for all key tiles + exp, then per head:
    PV (with a rowsum ones-column in V), 1/rowsum, normalize, then the
    output projection.
  - QK^T matmuls and exp sliced to the causal region of the 4 diagonal
    key-tiles; causal masking multiplies exp by a tril pattern on the
    gpsimd engine (off the DVE).
  - 1/rowsum as exp(-ln(rowsum)) on the ACT engine: a [1,512] DVE
    reciprocal is ~6 cycles/elem on one lane (~3.3us); ln+exp is 2x~700ns,
    and sits in the ACT queue right after the block's own exps, so the
    PE-side broadcast never waits behind the next block's exp batch.
"""

import numpy as np

import bass_rust
import concourse.bass as bass
import concourse.mybir as mybir
import concourse.tile as tile
from concourse.bass_utils import run_bass_kernel_spmd

F16 = mybir.dt.float16
F32 = mybir.dt.float32

B, S, D = 2, 2048, 768
H, DK = 12, 64
HPC = 3            # heads per core
N_CORES = 8
QB = 512           # query block (psum free dim)
NQB = S // QB      # 4
NKT = S // 128     # 16 key tiles
DKT = D // 128     # 6 contraction tiles for projections
NST = S // 128     # 16 s-chunks

ScopedClock = bass_rust.ScopedClock


# ---------------------------------------------------------------------------
# walrus in this build accepts at most ONE sync-wait per instruction; spread
# extra waits onto NOPs placed immediately before the owning instruction.

def _split_drain_and_barrier(self, tick_clock, wait_clock):
    probe = self.nc.sync.nop()
    wait_clock.add_sem_waits(probe.ins, ScopedClock({None: tick_clock.global_clock}))
    si = probe.ins.sync_info
    waits = list(si.on_wait) if si is not None else []
    if len(waits) > 1:
        si.on_wait = waits[:1]
        for w in waits[1:]:
            n = self.nc.sync.nop()
            nsi = n.ins.sync_info
            if nsi is None:
                n.ins.sync_info = bass_rust.SyncInfo(on_wait=[w], on_update=[])
            else:
                nsi.on_wait = [w]
    self.nc.sync.drain()

    self.nc.all_engine_barrier()
    assert self.sems is not None
    popped = self.nc._tile_sem_poison_stack.pop()
    assert popped is self._sem_poison
    self.nc.clear_and_free_semaphores(list(self.sems.allocated().values()))
    self.nc.all_engine_barrier()


tile.TileContext._drain_and_barrier = _split_drain_and_barrier

_nop_ctr = [0]


def split_multi_waits(nc):
    def visit(parent):
        for bb in parent.blocks:
            insts = bb.instructions
            out = []
            changed = False
            for inst in insts:
                si = inst.sync_info
                if si is not None and len(si.on_wait) > 1:
                    waits = list(si.on_wait)
                    for w in waits[:-1]:
                        _nop_ctr[0] += 1
                        nop = mybir.InstNoOp(
                            name=f"wsplit{_nop_ctr[0]}",
                            sync_info=mybir.SyncInfo(on_wait=[w], on_update=[]),
                            bass_nofuse=True,
                            engine=inst.engine,
                        )
                        out.append(nop)
                    si.on_wait = waits[-1:]
                    changed = True
                out.append(inst)
            if changed:
                bb.instructions = out
    for f in nc.m.functions:
        visit(f)


# ---------------------------------------------------------------------------


FEAT = {"lnexp", "gpmul", "slice", "warm"}


def build_nc(causal: bool, timing: bool = False, nrep: int = 1, feat=None):
    feat = FEAT if feat is None else feat
    nc = bass.Bass("TRN2", target_bir_lowering=False, debug=False,
                   num_devices=N_CORES)

    # x shipped chunk-major so each per-block DMA is contiguous per partition
    xt16 = nc.dram_tensor("xt16", (NQB, 128, DKT, QB), F16, kind="ExternalInput").ap()
    wqke = nc.dram_tensor("wqke", (128, DKT, 384), F16, kind="ExternalInput").ap()
    wve = nc.dram_tensor("wve", (128, DKT, HPC * DK), F16, kind="ExternalInput").ap()
    woh2 = nc.dram_tensor("woh2", (DK, D), F16, kind="ExternalInput").ap()
    woe2 = nc.dram_tensor("woe2", (128, D), F16, kind="ExternalInput").ap()
    bqk = nc.dram_tensor("bqk", (128, 3), F32, kind="ExternalInput").ap()
    if not causal:
        mTd = nc.dram_tensor("mT", (S, S), F16, kind="ExternalInput").ap()
    if timing:
        o16 = nc.dram_tensor("o16", (S, D), F16, kind="Internal").ap()
        dummy = nc.dram_tensor("tout", (128, 4), F32, kind="ExternalOutput").ap()
    else:
        o16 = nc.dram_tensor("o16", (S, D), F16, kind="ExternalOutput").ap()

    with tile.TileContext(nc) as tc:
        cst = tc.alloc_tile_pool(name="cst", bufs=1)
        ps_gen = tc.alloc_tile_pool(name="psg", bufs=2, space="PSUM")
        ps_sc = tc.alloc_tile_pool(name="pss", bufs=2, space="PSUM")
        ps_pv = tc.alloc_tile_pool(name="psv", bufs=2, space="PSUM")
        pt_pool = tc.alloc_tile_pool(name="ptp", bufs=2)
        ctx_pool = tc.alloc_tile_pool(name="ctp", bufs=2)
        rs_pool = tc.alloc_tile_pool(name="rsp", bufs=2)
        out_pool = tc.alloc_tile_pool(name="outp", bufs=3)
        if not causal:
            msk_pool = tc.alloc_tile_pool(name="mskp", bufs=2)

        for _rep in range(nrep):
            # ---- on-chip constants first (no DMA dependence)
            tril = cst.tile([128, 128], F16, tag="tril")
            nc.gpsimd.memset(tril[:], 1.0)
            # keep (f - p >= 0) i.e. q >= k, else 0
            nc.gpsimd.affine_select(
                out=tril[:], in_=tril[:], compare_op=mybir.AluOpType.is_ge,
                fill=0.0, base=0, pattern=[[1, 128]], channel_multiplier=-1)

            ones_bc = cst.tile([128, DK], F32, tag="ones")
            nc.vector.memset(ones_bc[:], 1.0)
            if timing:
                nc.sync.dma_start(dummy, ones_bc[:, 0:4])

            # ---- DMA: small weights first, then x in per-block chunks so the
            # first projection can start early
            wqk_sb = cst.tile([128, DKT, 384], F16, tag="wqk")
            wv_sb = cst.tile([128, DKT, HPC * DK], F16, tag="wv")
            wo2_sb = cst.tile([DK, D], F16, tag="woh2")
            wo_sb2 = cst.tile([128, D], F16, tag="wo2")
            bqk_sb = cst.tile([128, 3], F32, tag="bqk")
            nc.sync.dma_start(wqk_sb[:], wqke)
            nc.sync.dma_start(wv_sb[:], wve)
            nc.sync.dma_start(bqk_sb[:], bqk)
            nc.sync.dma_start(wo_sb2[:], woe2)
            nc.sync.dma_start(wo2_sb[:], woh2)
            xts = []
            for sb in range(NQB):
                xt = cst.tile([128, DKT, QB], F16, tag=f"xt{sb}", name=f"xt{sb}")
                nc.sync.dma_start(xt[:], xt16[sb])
                xts.append(xt)

            # ---- PE warm-up: dummy matmuls on an on-chip constant keep the
            # HAM activity window busy during the DMA fill so the real stream
            # starts at the full 2.4 GHz clock
            if "warm" in feat:
                wtile = cst.tile([128, 512], F16, tag="warm")
                nc.gpsimd.memset(wtile[:], 0.0)
                for w in range(8):
                    wps = ps_gen.tile([128, QB], F32, tag="psg")
                    nc.tensor.matmul(wps[:], wtile[:, 0:128], wtile[:],
                                     start=True, stop=True)

            # per-block q/k tiles: chunk 0 = [h0q|h1q], 1 = [h0k|h1k],
            # 2 = [h2q|h2k]; k2a = h2k re-aligned to partitions 0:64 so the
            # h2 QK^T operands share a partition range
            qk = [[cst.tile([128, QB], F16, tag=f"qk{ci}_{sb}",
                            name=f"qk{ci}_{sb}") for sb in range(NQB)]
                  for ci in range(3)]
            k2a = cst.tile([DK, S], F16, tag="k2a")
            # per-block V tiles [128, 4(st), HPC, DK+1]
            Vt = []
            for sb in range(NQB):
                V = cst.tile([128, 4, HPC, DK + 1], F16, tag=f"V{sb}",
                             name=f"V{sb}")
                nc.vector.memset(V[:, :, :, DK:DK + 1], 1.0)  # rowsum ones col
                Vt.append(V)

            def proj_block(sb):
                for ci in range(3):
                    ps = ps_gen.tile([128, QB], F32, tag="psg")
                    for c in range(DKT):
                        nc.tensor.matmul(
                            ps[:], wqk_sb[:, c, ci * 128:(ci + 1) * 128],
                            xts[sb][:, c, :],
                            start=(c == 0), stop=(c == DKT - 1))
                    nc.vector.tensor_scalar_add(qk[ci][sb][:], ps[:],
                                                bqk_sb[:, ci:ci + 1])
                nc.sync.dma_start(k2a[0:DK, sb * QB:(sb + 1) * QB],
                                  qk[2][sb][DK:128, :])
                for stl in range(4):
                    ps = ps_gen.tile([128, QB], F32, tag="psg")
                    for c in range(DKT):
                        nc.tensor.matmul(
                            ps[:, 0:HPC * DK],
                            xts[sb][:, c, stl * 128:(stl + 1) * 128],
                            wv_sb[:, c, :], start=(c == 0), stop=(c == DKT - 1))
                    nc.vector.tensor_copy(
                        Vt[sb][:, stl, :, 0:DK],
                        ps[:, 0:HPC * DK].rearrange("p (h d) -> p h d", d=DK))

            def off_of(i, j):
                return 128 * (j - 4 * i) if (causal and j >= 4 * i) else 0

            def qk_off(i, j):
                return off_of(i, j) if "slice" in feat else 0

            st_ctx = {}
            outq = []   # pending output-projection emitters (PE filler work)

            def emit_filler(n):
                for _ in range(n):
                    if outq:
                        outq.pop(0)()

            def qk_pair(i, sc, pts01, g2, mtile):
                """QK^T for heads h0+h1, key-tiles 2*g2 and 2*g2+1."""
                offs = []
                for jj in range(2):
                    j = 2 * g2 + jj
                    off = qk_off(i, j)
                    offs.append(off)
                    for h, rows in ((0, slice(0, DK)), (1, slice(DK, 128))):
                        nc.tensor.matmul(
                            sc[h][:, jj, off:QB],
                            qk[1][j // 4][rows, (j % 4) * 128:(j % 4 + 1) * 128],
                            qk[0][i][rows, off:QB],
                            start=True, stop=True, tile_position=(rows.start, 0))
                o0 = offs[0]
                for h in range(2):
                    nc.scalar.activation(
                        pts01[h][:, 2 * g2:2 * g2 + 2, o0:QB],
                        sc[h][:, :, o0:QB], mybir.ActivationFunctionType.Exp)
                for jj in range(2):
                    j = 2 * g2 + jj
                    if causal and j >= 4 * i:
                        off = 128 * (j - 4 * i)
                        for h in range(2):
                            nc.gpsimd.tensor_mul(
                                pts01[h][:, j, off:off + 128],
                                pts01[h][:, j, off:off + 128], tril[:])
                    elif not causal:
                        for h in range(2):
                            nc.gpsimd.tensor_mul(
                                pts01[h][:, j, :], pts01[h][:, j, :],
                                mtile[:, j, :])

            def qk_pair_h2(i, sc, pt2, g2, mtile):
                offs = []
                for jj in range(2):
                    j = 2 * g2 + jj
                    off = qk_off(i, j)
                    offs.append(off)
                    nc.tensor.matmul(
                        sc[:, jj, off:QB],
                        k2a[0:DK, j * 128:(j + 1) * 128],
                        qk[2][i][0:DK, off:QB],
                        start=True, stop=True, tile_position=(0, 0))
                o0 = offs[0]
                nc.scalar.activation(
                    pt2[:, 2 * g2:2 * g2 + 2, o0:QB], sc[:, :, o0:QB],
                    mybir.ActivationFunctionType.Exp)
                for jj in range(2):
                    j = 2 * g2 + jj
                    if causal and j >= 4 * i:
                        off = 128 * (j - 4 * i)
                        nc.gpsimd.tensor_mul(
                            pt2[:, j, off:off + 128],
                            pt2[:, j, off:off + 128], tril[:])
                    elif not causal:
                        nc.gpsimd.tensor_mul(
                            pt2[:, j, :], pt2[:, j, :], mtile[:, j, :])

            def pv_mm(i, pv, pt, h, j, kt):
                off = off_of(i, j)
                nc.tensor.matmul(
                    pv[0:DK + 1, off:QB], Vt[j // 4][:, j % 4, h, :],
                    pt[:, j, off:QB], start=(j == 0), stop=(j == kt - 1),
                    skip_group_check=True)

            def lnexp(pv):
                # 1/rowsum as exp(-ln(rowsum)) on the ACT engine -- a [1,512]
                # DVE reciprocal costs ~3.3us (one lane, ~6 cycles/elem)
                rs = rs_pool.tile([128, QB], F32, tag="rs", name=nc.get_next_instruction_name() + "_rs")
                rs2 = rs_pool.tile([128, QB], F32, tag="rs2", name=nc.get_next_instruction_name() + "_rs2")
                nc.scalar.activation(rs[DK:DK + 1, :], pv[DK:DK + 1, :],
                                     mybir.ActivationFunctionType.Ln)
                nc.scalar.activation(rs2[DK:DK + 1, :], rs[DK:DK + 1, :],
                                     mybir.ActivationFunctionType.Exp,
                                     scale=-1.0)
                return rs2

            def norm(i, h, pv, rs2, bc01):
                ctx, c01 = st_ctx[i]
                dst = c01[0:DK, :] if h == 0 else ctx[:, h, :]
                # h0 -> bc01 PE cols 0:64, h1 -> cols 64:128; h2 own tile
                if h < 2:
                    bc = bc01[h * DK:(h + 1) * DK, :]
                    tp = (DK, h * DK)
                else:
                    bch2 = ps_gen.tile([128, QB], F32, tag="psg")
                    bc = bch2[0:DK, :]
                    tp = (DK, 0)
                nc.tensor.matmul(bc, ones_bc[DK:DK + 1, 0:DK],
                                 rs2[DK:DK + 1, :], start=True, stop=True,
                                 tile_position=tp)
                bcs = rs_pool.tile([DK, QB], F32, tag="bcs")
                nc.vector.tensor_copy(bcs[:], bc)
                nc.vector.tensor_mul(dst, pv[0:DK, :], bcs[:])

            def queue_outproj(i):
                ctx, c01 = st_ctx[i]

                def mk(cch):
                    def emit():
                        chunk = i * (QB // 128) + cch
                        csl = slice(cch * 128, (cch + 1) * 128)
                        osb = out_pool.tile([128, D], F16, tag="osb")
                        for nb, ncols in ((0, 512), (512, 256)):
                            ps = ps_gen.tile([128, QB], F32, tag="psg")
                            nc.tensor.matmul(ps[:, 0:ncols], c01[:, csl],
                                             wo_sb2[:, nb:nb + ncols],
                                             start=True, stop=False)
                            nc.tensor.matmul(ps[:, 0:ncols], ctx[:, 2, csl],
                                             wo2_sb[:, nb:nb + ncols],
                                             start=False, stop=True)
                            nc.vector.tensor_copy(osb[:, nb:nb + ncols],
                                                  ps[:, 0:ncols])
                        nc.sync.dma_start(o16[chunk * 128:(chunk + 1) * 128, :],
                                          osb[:])
                    return emit
                for cch in range(QB // 128):
                    outq.append(mk(cch))

            def attn_block(i):
                kt = 4 * (i + 1) if causal else NKT
                mtile = None
                if not causal:
                    mtile = msk_pool.tile([128, NKT, QB], F16, tag="mt")
                    nc.sync.dma_start(
                        mtile[:],
                        mTd.rearrange("(kt p) q -> p kt q", p=128)[:, :, i * QB:(i + 1) * QB])
                ctx = ctx_pool.tile([DK, HPC, QB], F16, tag="ctx")
                c01 = ctx_pool.tile([128, QB], F16, tag="c01")
                st_ctx[i] = (ctx, c01)
                pts = [pt_pool.tile([128, NKT, QB], F16, tag=f"pt{h}",
                                    name=f"pt{h}_{i}") for h in range(HPC)]
                # QK^T + exp for all heads of the block (batched phases, v2
                # order -- keeps the PE stream dense and the HAM warm)
                for g2 in range(kt // 2):
                    sc = [ps_sc.tile([128, 2, QB], F32, tag="sc",
                                     name=f"scp{i}_{g2}_{k}") for k in range(2)]
                    qk_pair(i, sc, pts, g2, mtile)
                for g2 in range(kt // 2):
                    sc2 = ps_sc.tile([128, 2, QB], F32, tag="sc",
                                     name=f"sch2_{i}_{g2}")
                    qk_pair_h2(i, sc2, pts[2], g2, mtile)
                if "fill" in feat:
                    emit_filler(4)
                bc01 = ps_gen.tile([128, QB], F32, tag="psg")
                for h in range(HPC):
                    pv = ps_pv.tile([128, QB], F32, tag="pv", name=f"pv{h}_{i}")
                    for j in range(kt):
                        pv_mm(i, pv, pts[h], h, j, kt)
                    rs2 = lnexp(pv)
                    norm(i, h, pv, rs2, bc01)
                # h1 ctx shifted to partitions 64-127 via a small on-chip DMA
                # (DVE cannot cross partitions) -> 128-row contraction below
                nc.sync.dma_start(c01[DK:128, :], ctx[:, 1, :])
                queue_outproj(i)
                if "fill" not in feat:
                    emit_filler(4)

            for sb in range(NQB):
                proj_block(sb)
            for i in range(NQB):
                attn_block(i)
            emit_filler(len(outq))

        pools = [cst, ps_gen, ps_sc, ps_pv, pt_pool, ctx_pool, rs_pool, out_pool]
        if not causal:
            pools.append(msk_pool)
        for p in reversed(pools):
            p.release()

    split_multi_waits(nc)
    return nc


_CACHE = {}


def _get_nc(causal):
    if causal not in _CACHE:
        _CACHE[causal] = build_nc(causal)
    return _CACHE[causal]


def _core_inputs(x, mask, wq, bq, wk, bk, wv, bv, wo, causal):
    ins = []
    if not causal:
        mT = (mask[0, 0].T != 0).astype(np.float16)
    for core in range(N_CORES):
        b, g = divmod(core, 4)
        hs = [HPC * g + k for k in range(HPC)]
        cols = lambda w, h: w[:, h * DK:(h + 1) * DK]

        # f16 q|k weight chunks: [h0q|h1q], [h0k|h1k], [h2q|h2k];
        # q columns carry the 1/8 softmax scale
        wq8 = wq * 0.125
        wqk = np.concatenate(
            [cols(wq8, hs[0]), cols(wq8, hs[1]),
             cols(wk, hs[0]), cols(wk, hs[1]),
             cols(wq8, hs[2]), cols(wk, hs[2])],
            axis=1).astype(np.float16)                          # (768, 384)
        wqk = np.ascontiguousarray(
            wqk.reshape(DKT, 128, 384).transpose(1, 0, 2))      # (128, 6, 384)

        xtf = x[b].T.reshape(DKT, 128, S).transpose(1, 0, 2)   # (128, 6, 2048)
        xt = np.ascontiguousarray(
            xtf.reshape(128, DKT, NQB, QB).transpose(2, 0, 1, 3))  # (4,128,6,512)

        wve = np.concatenate([wv[:, h * DK:(h + 1) * DK] for h in hs],
                             axis=1).astype(np.float16)          # (768, 192)
        wve = np.ascontiguousarray(
            wve.reshape(DKT, 128, HPC * DK).transpose(1, 0, 2))  # (128, 6, 192)

        woh2 = wo[hs[2] * DK:(hs[2] + 1) * DK].astype(np.float16)
        woe2 = np.concatenate([wo[hs[0] * DK:(hs[0] + 1) * DK],
                               wo[hs[1] * DK:(hs[1] + 1) * DK]]).astype(np.float16)
        seg = lambda v, h, s=1.0: v[h * DK:(h + 1) * DK] * s
        bqk_pack = np.stack([
            np.concatenate([seg(bq, hs[0], 0.125), seg(bq, hs[1], 0.125)]),
            np.concatenate([seg(bk, hs[0]), seg(bk, hs[1])]),
            np.concatenate([seg(bq, hs[2], 0.125), seg(bk, hs[2])]),
        ], axis=1).astype(np.float32)

        m = {
            "xt16": xt.astype(np.float16),
            "wqke": wqk, "wve": wve, "woh2": woh2, "woe2": woe2,
            "bqk": bqk_pack,
        }
        if not causal:
            m["mT"] = mT
        ins.append(m)
    return ins


def kernel(x, mask, wq, bq, wk, bk, wv, bv, wo, bo):
    x = np.asarray(x)
    mask = np.asarray(mask)
    m2 = np.asarray(mask[0, 0])
    causal = bool(np.array_equal(m2, np.tril(np.ones((S, S), m2.dtype))))
    nc = _get_nc(causal)
    ins = _core_inputs(x, mask, np.asarray(wq), np.asarray(bq), np.asarray(wk),
                       np.asarray(bk), np.asarray(wv), np.asarray(bv),
                       np.asarray(wo), causal)
    res = run_bass_kernel_spmd(nc, ins, core_ids=list(range(N_CORES)))
    out = np.zeros((B, S, D), np.float32)
    for core in range(N_CORES):
        b = core // 4
        out[b] += res.results[core]["o16"].astype(np.float32)
    # exact host-side fold: v-bias rides through attention as a constant
    out += np.asarray(bo, np.float32) + np.asarray(bv, np.float32) @ np.asarray(wo, np.float32)
    return out
